# revision 23
# baseline (speedup 1.0000x reference)
"""Self-contained Trainium2 kernel for nn_Attention_58033598104213.

GQA causal attention block (B=2, T=2048, d_model=2048, 16 Q heads / 4 KV
heads, head_dim=128, RoPE, causal SDPA, output projection).

Sharding: 8 NeuronCores = 2 batches x 4 head-groups. Core (b, g) computes
all T queries of batch b for Q-heads 4g..4g+3 (which share KV head g) and
the partial product against wo's matching row slice; the host sums the 4
partials per batch (row-parallel wo => partial-sum gather). No collectives.

v2: fully fused per-512-token-chunk pipeline. For chunk m: QKV projection
(+RoPE) -> causal flash attention over key blocks 0..4m+3 in head pairs
-> output projection of chunk m-1 interleaved into the attention rounds as
PE filler work. Softmax denominators: bf16 quad-tree block sums on DVE,
partition all-reduce + broadcast on GpSimd, reciprocal on DVE. V transposed
via XBAR DMA transpose. All compute bf16 with fp32 PSUM accumulation.
"""
import numpy as np
import ml_dtypes
import orjson
import concourse.bass as bass
import concourse.mybir as mybir
import concourse.tile as tile
from concourse import bass_isa, library_config
from concourse.bass_utils import run_bass_kernel_spmd

# ---------------------------------------------------------------------------
# Walrus in this image accepts only one sem-wait per instruction; the Tile
# framework's final drain carries several. Split excess waits onto preceding
# NoOps on the same engine (in-order execution preserves the AND semantics).
_MARK = "_bir_wait_split_patched"


def split_waits(bir: bytes, maxw: int = 1) -> bytes:
    m = orjson.loads(bir)
    n_split = 0

    def fix_instructions(insts: list) -> list:
        nonlocal n_split
        out = []
        for ins in insts:
            si = ins.get("sync_info")
            waits = (si or {}).get("on_wait") or []
            if len(waits) > maxw:
                n_split += 1
                head, rest = waits[: len(waits) - maxw], waits[len(waits) - maxw :]
                for k in range(0, len(head), maxw):
                    out.append(
                        {
                            "debug": ins.get("debug", 0),
                            "engine": ins["engine"],
                            "ins": [],
                            "name": f"{ins['name']}-wsplit{k}",
                            "opcode": "NoOp",
                            "outs": [],
                            "sync_info": {
                                "on_update": [],
                                "on_wait": head[k : k + maxw],
                            },
                        }
                    )
                si["on_wait"] = rest
            out.append(ins)
        return out

    def walk(o):
        if isinstance(o, dict):
            if isinstance(o.get("instructions"), list):
                o["instructions"] = fix_instructions(o["instructions"])
            for v in o.values():
                walk(v)
        elif isinstance(o, list):
            for v in o:
                walk(v)

    walk(m)
    return orjson.dumps(m)


def patch_nc(nc, maxw: int = 1):
    if getattr(nc, _MARK, False):
        return nc
    orig = nc.to_json_bytes

    def wrapped(*a, **kw):
        return split_waits(orig(*a, **kw), maxw=maxw)

    nc.to_json_bytes = wrapped
    setattr(nc, _MARK, True)
    return nc


# ---------------------------------------------------------------------------
F32 = mybir.dt.float32
F32R = mybir.dt.float32r
BF16 = mybir.dt.bfloat16
AF = mybir.ActivationFunctionType
OP = None  # filled lazily to avoid import issues

P = 128
T = 2048
D = 2048
NT = D // P  # 16 d-blocks of 128
HQ = 4  # Q heads per core
HD = 128
CH = 512  # token chunk
NCH = T // CH  # 4
KB = 128  # key block
HALF = HD // 2
SCALE = float(1.0 / np.sqrt(HD))

# use GpSimd for softmax partition reduce/broadcast (else PE ones-matmuls).
# The gpsimd custom-ISA ops fail walrus codegen in this image ("ISA wrong
# length"), so the PE path is the working one.
GPSIMD_DENOM = False


def _r(ap):
    return ap.bitcast(F32R)


def build():
    from concourse.alu_op_type import AluOpType

    nc = bass.Bass()
    xT = nc.declare_dram_parameter("xT", [P, NCH * NT * CH], BF16, isOutput=False)
    wq = nc.declare_dram_parameter("wq", [P, NT * HQ * HD], BF16, isOutput=False)
    wk = nc.declare_dram_parameter("wk", [P, NT * HD], BF16, isOutput=False)
    wv = nc.declare_dram_parameter("wv", [P, NT * HD], BF16, isOutput=False)
    wo = nc.declare_dram_parameter("wo", [P, HQ * D], BF16, isOutput=False)
    cos2d = nc.declare_dram_parameter("cos2", [P, T], BF16, isOutput=False)
    sin2d = nc.declare_dram_parameter("sin2", [P, T], BF16, isOutput=False)
    maskp = nc.declare_dram_parameter("maskp", [P, CH], BF16, isOutput=False)
    ones_c_d = nc.declare_dram_parameter("ones_c", [P, 1], F32, isOutput=False)
    ones_r_d = nc.declare_dram_parameter("ones_r", [1, P], BF16, isOutput=False)
    out = nc.declare_dram_parameter("out", [T, D], BF16, isOutput=True)

    xT_t = xT.rearrange("p (m t n) -> p m t n", t=NT, n=CH)
    wk_t = wk.rearrange("p (t n) -> p t n", t=NT)
    wv_t = wv.rearrange("p (t n) -> p t n", t=NT)
    wq_t = wq.rearrange("p (t n) -> p t n", t=NT)
    wo_t = wo.rearrange("p (h n) -> p h n", h=HQ)

    with (
        tile.TileContext(nc) as tc,
        nc.allow_low_precision(reason="bf16 compute"),
        tc.tile_pool(name="res", bufs=1) as res,
        tc.tile_pool(name="xtp", bufs=2) as xtp,
        tc.tile_pool(name="qcx", bufs=2) as qcx,
        tc.tile_pool(name="wrk", bufs=3) as wrk,
        tc.tile_pool(name="ptp", bufs=3) as ptp,
        tc.tile_pool(name="pp", bufs=2, space="PSUM") as pp,
        tc.tile_pool(name="pss", bufs=3, space="PSUM") as pss,
        tc.tile_pool(name="psc", bufs=3, space="PSUM") as psc,
    ):
        # ---- residents
        kT_s = res.tile([P, T], BF16, tag="kT", name="kT")
        v_s = res.tile([P, T // P, HD], BF16, tag="v", name="v")
        wq_s = res.tile([P, NT, HQ * HD], BF16, tag="wq", name="wq_s")
        wk_s = res.tile([P, NT, HD], BF16, tag="wk", name="wk_s")
        wv_s = res.tile([P, NT, HD], BF16, tag="wv", name="wv_s")
        wo_s = res.tile([P, HQ, D], BF16, tag="wo", name="wo_s")
        cos_s = res.tile([P, T], BF16, tag="cos", name="cos_s")
        sin_s = res.tile([P, T], BF16, tag="sin", name="sin_s")
        mask_s = res.tile([P, CH], BF16, tag="mask", name="mask_s")
        ones_c = res.tile([P, 1], F32, tag="ones_c", name="ones_c")
        ones_r = res.tile([1, P], BF16, tag="ones_r", name="ones_r")

        if GPSIMD_DENOM:
            nc.gpsimd.load_library(library_config.attn)

        # ---- preamble DMAs, ordered so the first matmul chain starts ASAP
        # (wk/xt0 split fine so the pk accumulation streams behind the DMA)
        nc.sync.dma_start(out=wk_s[:, 0 : NT // 2, :], in_=wk_t[:, 0 : NT // 2, :])
        xt0 = xtp.tile([P, NT, CH], BF16, tag="xt", name="xt0")
        nc.sync.dma_start(out=xt0[:, 0:4, :], in_=xT_t[:, 0, 0:4, :])
        nc.sync.dma_start(out=wk_s[:, NT // 2 :, :], in_=wk_t[:, NT // 2 :, :])
        nc.sync.dma_start(out=xt0[:, 4:8, :], in_=xT_t[:, 0, 4:8, :])
        nc.sync.dma_start(out=cos_s[:], in_=cos2d[:])
        nc.sync.dma_start(out=xt0[:, 8:12, :], in_=xT_t[:, 0, 8:12, :])
        nc.sync.dma_start(out=sin_s[:], in_=sin2d[:])
        nc.sync.dma_start(out=xt0[:, 12:16, :], in_=xT_t[:, 0, 12:16, :])
        nc.sync.dma_start(out=wv_s[:], in_=wv_t)
        nc.sync.dma_start(out=mask_s[:], in_=maskp[:])
        nc.sync.dma_start(out=_r(ones_c[:]), in_=ones_c_d[:].bitcast(F32R))
        nc.sync.dma_start(out=ones_r[:], in_=ones_r_d[:])
        nc.sync.dma_start(out=wq_s[:, 0 : NT // 2, :], in_=wq_t[:, 0 : NT // 2, :])
        nc.sync.dma_start(out=wq_s[:, NT // 2 :, :], in_=wq_t[:, NT // 2 :, :])

        xts = {0: xt0}
        qTs = {}  # (m, h) -> tile (current chunk only)
        ctxs = {}  # (m, h) -> tile
        pending = []  # deferred per-pair denominator tails

        def flush_pending():
            while pending:
                pending.pop(0)()

        def rope_evict(m, p, dst_full, dst_sl, uid):
            """dst_full[:, dst_sl] = bf16 rope(p) for chunk m's positions;
            p is fp32 PSUM [P, CH]."""
            tab = slice(m * CH, (m + 1) * CH)  # rope table = token positions
            pb = wrk.tile([P, CH], BF16, tag="pb", name=f"pb{uid}")
            nc.scalar.copy(pb[:], p[:])  # ACT: PSUM fp32 -> SBUF bf16
            sw = wrk.tile([P, CH], BF16, tag="sw", name=f"sw{uid}")
            nc.vector.tensor_copy(sw[0:HALF, :], pb[HALF:P, :])
            nc.vector.tensor_copy(sw[HALF:P, :], pb[0:HALF, :])
            t2 = wrk.tile([P, CH], BF16, tag="t2", name=f"t2{uid}")
            nc.vector.tensor_mul(t2[:], sw[:], sin_s[:, tab])
            t3 = wrk.tile([P, CH], BF16, tag="t3", name=f"t3{uid}")
            nc.vector.tensor_mul(t3[:], pb[:], cos_s[:, tab])
            nc.vector.tensor_add(dst_full[:, dst_sl], t3[:], t2[:])

        def proj_chunk(m):
            flush_pending()
            sl = slice(m * CH, (m + 1) * CH)
            xt = xts[m]
            # K projection + rope
            pk = pp.tile([P, CH], F32, tag="mm", name=f"pk{m}")
            for t in range(NT):
                nc.tensor.matmul(
                    pk[:], wk_s[:, t, :], xt[:, t, :],
                    start=(t == 0), stop=(t == NT - 1),
                )
            rope_evict(m, pk, kT_s, sl, f"k{m}")
            # V projection + XBAR transpose into v_s
            pv = pp.tile([P, CH], F32, tag="mm", name=f"pv{m}")
            for t in range(NT):
                nc.tensor.matmul(
                    pv[:], wv_s[:, t, :], xt[:, t, :],
                    start=(t == 0), stop=(t == NT - 1),
                )
            vt = wrk.tile([P, CH], BF16, tag="vt", name=f"vt{m}")
            nc.scalar.copy(vt[:], pv[:])
            nc.sync.dma_start_transpose(
                out=v_s[:, 4 * m : 4 * (m + 1), :], in_=vt[:]
            )
            # Q projections + rope
            for h in range(HQ):
                pq = pp.tile([P, CH], F32, tag="mm", name=f"pq{m}_{h}")
                for t in range(NT):
                    nc.tensor.matmul(
                        pq[:],
                        wq_s[:, t, h * HD : (h + 1) * HD],
                        xt[:, t, :],
                        start=(t == 0),
                        stop=(t == NT - 1),
                    )
                qT = qcx.tile([P, CH], BF16, tag=f"qT{h}", name=f"qT{m}_{h}")
                qTs[(m, h)] = qT
                rope_evict(m, pq, qT, slice(0, CH), f"q{m}_{h}")

        def oproj_group(m, u, n):
            """One output block: out[(4m+u)*128 : .. , n*512 : ..]."""
            po = pp.tile([P, CH], F32, tag="mm", name=f"po{m}_{u}_{n}")
            for h in range(HQ):
                nc.tensor.matmul(
                    po[:],
                    ctxs[(m, h)][:, u * P : (u + 1) * P],
                    wo_s[:, h, n * CH : (n + 1) * CH],
                    start=(h == 0),
                    stop=(h == HQ - 1),
                )
            so = wrk.tile([P, CH], BF16, tag="so", name=f"so{m}_{u}_{n}")
            if (u + n) % 2 == 0:
                nc.vector.tensor_copy(so[:], po[:])
            else:
                nc.scalar.copy(so[:], po[:])
            r0 = (4 * m + u) * P
            nc.sync.dma_start(
                out=out[r0 : r0 + P, n * CH : (n + 1) * CH], in_=so[:]
            )

        def attn_chunk(m, filler):
            nblk = 4 * (m + 1)
            n_rounds = 2 * nblk
            emitted = 0
            rnd = 0
            for pair in ((0, 1), (2, 3)):
                cps = {}
                sacc = {}
                pend = {}  # h -> list of pending bf16 pair tiles
                npair = {}  # h -> count of emitted pair adds
                nacc = {}  # h -> count of fp32 accumulations
                for h in pair:
                    cps[h] = psc.tile([P, CH], F32, tag="cp", name=f"cp{m}_{h}")
                    sacc[h] = wrk.tile(
                        [P, CH], F32, tag=f"sa{h % 2}", name=f"sa{m}_{h}"
                    )
                    pend[h] = []
                    npair[h] = 0
                    nacc[h] = 0
                prev_pT = {h: None for h in pair}  # last off-diag pT awaiting pair
                for j in range(nblk):
                    d = j - 4 * m
                    q0 = P * d if d >= 0 else 0
                    sps = {}
                    for h in pair:
                        sp = pss.tile([P, CH], F32, tag="sp", name=f"sp{m}_{h}_{j}")
                        nc.tensor.matmul(
                            sp[:, q0:CH],
                            kT_s[:, j * KB : (j + 1) * KB],
                            qTs[(m, h)][:, q0:CH],
                            start=True,
                            stop=True,
                        )
                        sps[h] = sp
                    pTs = {}
                    for h in pair:
                        pT = ptp.tile([P, CH], BF16, tag=f"pT{h}", name=f"pT{m}_{h}_{j}")
                        nc.scalar.activation(
                            pT[:, q0:CH], sps[h][:, q0:CH], AF.Exp, scale=SCALE
                        )
                        if d >= 0:
                            nc.vector.tensor_mul(
                                pT[:, q0:CH], pT[:, q0:CH], mask_s[:, 0 : CH - q0]
                            )
                        pTs[h] = pT
                    # deferred pair tails + filler (oproj of chunk m-1) sit
                    # between exp and PV so the PE has independent work while
                    # ACT produces pT
                    rnd += 1
                    npop = 0
                    while pending and npop < 2:
                        pending.pop(0)()
                        npop += 1
                    target = (rnd * len(filler)) // n_rounds
                    while emitted < target:
                        filler[emitted]()
                        emitted += 1
                    for h in pair:
                        nc.tensor.matmul(
                            cps[h][:, q0:CH],
                            v_s[:, j, :],
                            pTs[h][:, q0:CH],
                            start=(j == 0),
                            stop=(j == nblk - 1),
                        )
                    # denominator block sums on DVE
                    for h in pair:
                        if d >= 0:
                            # diagonal: direct fp32 accumulate on live slice
                            if nacc[h] == 0 and j == 0:
                                nc.vector.tensor_copy(_r(sacc[h][:]), pTs[h][:])
                                nacc[h] += 1
                            else:
                                nc.vector.tensor_add(
                                    _r(sacc[h][:, q0:CH]),
                                    sacc[h][:, q0:CH],
                                    pTs[h][:, q0:CH],
                                )
                        elif prev_pT[h] is None:
                            prev_pT[h] = pTs[h]
                        else:
                            pr = ptp.tile(
                                [P, CH], BF16, tag=f"tp{h % 2}", name=f"tp{m}_{h}_{j}"
                            )
                            nc.vector.tensor_add(pr[:], prev_pT[h][:], pTs[h][:])
                            prev_pT[h] = None
                            pend[h].append(pr)
                            npair[h] += 1
                            if len(pend[h]) == 2:
                                a, b = pend[h]
                                pend[h] = []
                                qd = ptp.tile(
                                    [P, CH], BF16, tag=f"qd{h % 2}",
                                    name=f"qd{m}_{h}_{j}",
                                )
                                nc.vector.tensor_add(qd[:], a[:], b[:])
                                if nacc[h] == 0:
                                    nc.vector.tensor_copy(_r(sacc[h][:]), qd[:])
                                else:
                                    nc.vector.tensor_add(
                                        _r(sacc[h][:]), sacc[h][:], qd[:]
                                    )
                                nacc[h] += 1
                # flush any leftover single pair (nblk % 4 == 0 keeps this empty,
                # but chunk 0 has all-diagonal blocks so pend stays empty anyway)
                for h in pair:
                    assert prev_pT[h] is None and not pend[h]
                # denominators: partition-reduce on PE (f32r ones-matmul) and
                # 1/x = exp(-ln x) on ACT, emitted eagerly; the broadcast
                # matmul + context eviction are deferred into the next rounds
                # (or next chunk) so the PE never idles on this chain.
                rrows = {}
                for h in pair:
                    sm = pp.tile([P, CH], F32, tag="mm", name=f"sm{m}_{h}")
                    nc.tensor.matmul(
                        sm[0:1, :], _r(ones_c[:]), _r(sacc[h][:]),
                        start=True, stop=True,
                    )
                    lns = wrk.tile([1, CH], F32, tag="lns", name=f"ln{m}_{h}")
                    nc.scalar.activation(lns[:], sm[0:1, :], AF.Ln)
                    rrow = wrk.tile([1, CH], BF16, tag="rrow", name=f"rr{m}_{h}")
                    nc.scalar.activation(rrow[:], lns[:], AF.Exp, scale=-1.0)
                    rrows[h] = rrow

                def tail(h, cp, rrow, m=m):
                    prb = pp.tile([P, CH], F32, tag="mm", name=f"prb{m}_{h}")
                    nc.tensor.matmul(
                        prb[:], ones_r[:], rrow[:], start=True, stop=True
                    )
                    rbc = wrk.tile([P, CH], F32, tag="rbc", name=f"rb{m}_{h}")
                    nc.scalar.copy(rbc[:], prb[:])
                    ctx = qcx.tile([P, CH], BF16, tag=f"cx{h}", name=f"cx{m}_{h}")
                    ctxs[(m, h)] = ctx
                    nc.vector.tensor_mul(ctx[:], cp[:], rbc[:])

                for h in pair:
                    pending.append(
                        lambda h=h, cp=cps[h], rrow=rrows[h]: tail(h, cp, rrow)
                    )
            # any unemitted filler
            while emitted < len(filler):
                filler[emitted]()
                emitted += 1

        # ---- the fused chunk pipeline
        for m in range(NCH):
            proj_chunk(m)
            if m + 1 < NCH:
                xt = xtp.tile([P, NT, CH], BF16, tag="xt", name=f"xt{m + 1}")
                xts[m + 1] = xt
                nc.sync.dma_start(
                    out=xt[:, 0 : NT // 2, :], in_=xT_t[:, m + 1, 0 : NT // 2, :]
                )
                nc.sync.dma_start(
                    out=xt[:, NT // 2 :, :], in_=xT_t[:, m + 1, NT // 2 :, :]
                )
            filler = []
            if m >= 1:
                for u in range(4):
                    for n in range(4):
                        filler.append(
                            lambda mm=m - 1, uu=u, nn=n: oproj_group(mm, uu, nn)
                        )
            attn_chunk(m, filler)
            if m == 0:
                # wo needed from chunk 1's attention rounds onward
                nc.sync.dma_start(out=wo_s[:, 0:2, :], in_=wo_t[:, 0:2, :])
                nc.sync.dma_start(out=wo_s[:, 2:4, :], in_=wo_t[:, 2:4, :])
        # tail: output projection of the final chunk
        flush_pending()
        for u in range(4):
            for n in range(4):
                oproj_group(NCH - 1, u, n)

    patch_nc(nc, maxw=1)
    return nc


# ---------------------------------------------------------------------------
def rope_tables():
    inv_freq = 1.0 / (10000.0 ** (np.arange(0, HD, 2, dtype=np.float64) / HD))
    t = np.arange(T, dtype=np.float64)
    freqs = np.outer(t, inv_freq)
    c = np.cos(freqs).T.astype(np.float32)
    s = np.sin(freqs).T.astype(np.float32)
    cos2 = np.concatenate([c, c], 0)
    sin2 = np.concatenate([-s, s], 0)
    bf = ml_dtypes.bfloat16
    return (
        np.ascontiguousarray(cos2.astype(bf)),
        np.ascontiguousarray(sin2.astype(bf)),
    )


def mask_pad():
    k = np.arange(P)[:, None]
    i = np.arange(CH)[None, :]
    return (i >= k).astype(ml_dtypes.bfloat16)


def _sbufify(w):
    """[NT*P, N] -> [P, NT*N]: row t*128+p lands at partition p, block t."""
    n = w.shape[1]
    return np.ascontiguousarray(
        w.reshape(NT, P, n).transpose(1, 0, 2).reshape(P, NT * n)
    )


def make_in_maps(x, wq, wk, wv, wo):
    bf = ml_dtypes.bfloat16
    cos2, sin2 = rope_tables()
    maskp = mask_pad()
    ones_c = np.ones((P, 1), np.float32)
    ones_r = np.ones((1, P), bf)
    # x[b]: [T, D] -> [P, NCH*NT*CH]: partition p, chunk m, d-block t holds
    # x[m*CH : (m+1)*CH, t*P+p]  (one contiguous line per chunk per partition)
    xps = []
    for b in range(2):
        xr = x[b].astype(bf).reshape(NCH, CH, NT, P).transpose(3, 0, 2, 1)
        xps.append(np.ascontiguousarray(xr.reshape(P, NCH * NT * CH)))
    wqb, wkb, wvb, wob = (a.astype(bf) for a in (wq, wk, wv, wo))
    in_maps = []
    for c in range(8):
        b, g = divmod(c, 4)
        wog = wob[512 * g : 512 * (g + 1)]  # [512, D]
        wopre = np.ascontiguousarray(
            wog.reshape(HQ, P, D).transpose(1, 0, 2).reshape(P, HQ * D)
        )
        in_maps.append(
            {
                "xT": xps[b],
                "wq": _sbufify(wqb[:, 512 * g : 512 * (g + 1)]),
                "wk": _sbufify(wkb[:, 128 * g : 128 * (g + 1)]),
                "wv": _sbufify(wvb[:, 128 * g : 128 * (g + 1)]),
                "wo": wopre,
                "cos2": cos2,
                "sin2": sin2,
                "maskp": maskp,
                "ones_c": ones_c,
                "ones_r": ones_r,
            }
        )
    return in_maps


def combine_outputs(results):
    out = np.zeros((2, T, D), np.float32)
    for c in range(8):
        out[c // 4] += results[c]["out"].astype(np.float32)
    return out


_NC_CACHE = []


def kernel(x, wq, wk, wv, wo):
    x = np.asarray(x, dtype=np.float32)
    wq = np.asarray(wq, dtype=np.float32)
    wk = np.asarray(wk, dtype=np.float32)
    wv = np.asarray(wv, dtype=np.float32)
    wo = np.asarray(wo, dtype=np.float32)
    if not _NC_CACHE:
        _NC_CACHE.append(build())
    nc = _NC_CACHE[0]
    in_maps = make_in_maps(x, wq, wk, wv, wo)
    res = run_bass_kernel_spmd(nc, in_maps, core_ids=list(range(8)))
    return combine_outputs(res.results)
